# revision 15
# baseline (speedup 1.0000x reference)
"""Sparse (sliding-window) attention Trainium2 kernel.

Problem (hardcoded shapes): B=32, N=1024 tokens on a 16x64 grid, C=256,
8 heads, head_dim=32. Local attention window: +-3 grid rows, +-5 grid
cols (7x11). y = softmax(q k^T/sqrt(d) + mask) v, projected.

Sharding: data-parallel over batch, 4 items per core on 8 cores.

Per-core algorithm (bf16 compute, fp32 PSUM accumulation):
  - qkvT[768,1024] = (w_qkv.T).T @ x.T via PE (host passes xT, w_qkv.T
    with the q part pre-scaled by d^-0.5). Layout keeps q/k per head at
    partition offsets 32j, which feeds the row-packed score matmuls.
  - scores in transposed layout ST[k_chunk=128, q_band<=512] per head:
    4 heads computed concurrently via tile_position row packing (K=32).
  - P = exp(ST) on ScalarE (PSUM->SBUF bf16, one op per 4-head group),
    then multiplied by a compact 0/1 band mask on VectorE (2x bf16).
  - out.T[d,q] and denominators accumulate per q-tile in PSUM: PV uses
    col-packed matmuls (lhsT = V chunk [128,32]); the denominator uses
    lhsT = ones [128,32], which lands the row-sum broadcast across the
    32 partitions of each head. PV and denominator live in different
    PSUM banks (start=True clears the whole bank for the written
    partitions, so they must not share one).
  - proj consumes the transposed attention output directly as lhsT.
"""

import contextlib

import numpy as np
import ml_dtypes

import concourse.bass as bass
import concourse.bacc as bacc
import concourse.mybir as mybir
import concourse.tile as tile
from concourse import bass_utils
from concourse.masks import make_identity

F32 = mybir.dt.float32
BF16 = mybir.dt.bfloat16
AF = mybir.ActivationFunctionType

H_MAP, W_MAP = 16, 64
N_TOK = H_MAP * W_MAP            # 1024
DIM = 256
HEADS = 8
HDIM = 32
B_FULL = 32
N_CORES = 8
B_LOC = B_FULL // N_CORES        # 4
NCHUNK = N_TOK // 128            # 8 k-chunks (2 grid rows each)
NQT = N_TOK // 128               # 8 q-tiles
HALF = 512


def _qband(c):
    """Valid q range (start token, width) for k-chunk c (rows 2c, 2c+1)."""
    qlo = max(0, 2 * c - 3)
    qhi = min(H_MAP - 1, 2 * c + 4)
    return qlo * W_MAP, (qhi - qlo + 1) * W_MAP


def build_program(loop_n=1):
    nc = bacc.Bacc("TRN2", target_bir_lowering=False, debug=False)

    xt_d = nc.dram_tensor("xt", [B_LOC, DIM, N_TOK], BF16, kind="ExternalInput")
    wqkvT_d = nc.dram_tensor("wqkvT", [DIM, 3 * DIM], BF16, kind="ExternalInput")
    wpT_d = nc.dram_tensor("wpT", [DIM, DIM], BF16, kind="ExternalInput")
    bias_d = nc.dram_tensor("bias", [1, DIM], BF16, kind="ExternalInput")
    maskc_d = nc.dram_tensor("maskc", [NCHUNK, 128, 512], BF16, kind="ExternalInput")
    y_d = nc.dram_tensor("y", [B_LOC, N_TOK, DIM], F32, kind="ExternalOutput")

    xt = xt_d.ap()
    y = y_d.ap()

    with tile.TileContext(nc) as tc:
        with (
            tc.tile_pool(name="const", bufs=1) as const,
            tc.tile_pool(name="xtp", bufs=4) as xtp,
            tc.tile_pool(name="qkvp", bufs=12) as qkvp,
            tc.tile_pool(name="vp", bufs=20) as vp,
            tc.tile_pool(name="ptp", bufs=9) as ptp,
            tc.tile_pool(name="atp", bufs=4) as atp,
            tc.tile_pool(name="drp", bufs=2) as drp,
            tc.tile_pool(name="yp", bufs=4) as yp,
            tc.tile_pool(name="sc_ps", bufs=3, space="PSUM") as sc_ps,
            tc.tile_pool(name="od_ps", bufs=1, space="PSUM") as od_ps,
            tc.tile_pool(name="mm_ps", bufs=1, space="PSUM") as mm_ps,
        ):
            # ---- constants ----
            wqkv_sb = [const.tile([128, 3 * DIM], BF16, tag=f"wqkv{i}", name=f"wqkv{i}")
                       for i in range(2)]
            for i in range(2):
                nc.sync.dma_start(out=wqkv_sb[i], in_=wqkvT_d.ap()[128 * i:128 * (i + 1), :])
            wp_sb = [const.tile([128, DIM], BF16, tag=f"wp{i}", name=f"wp{i}")
                     for i in range(2)]
            for i in range(2):
                nc.sync.dma_start(out=wp_sb[i], in_=wpT_d.ap()[128 * i:128 * (i + 1), :])
            bias_sb = const.tile([1, DIM], BF16, tag="bias", name="bias_sb")
            nc.sync.dma_start(out=bias_sb, in_=bias_d.ap())
            mask_sb = [const.tile([128, 512], BF16, tag=f"mask{c}", name=f"mask{c}")
                       for c in range(NCHUNK)]
            for c in range(NCHUNK):
                nc.sync.dma_start(out=mask_sb[c], in_=maskc_d.ap()[c])
            ones32 = const.tile([128, 32], BF16, tag="ones32", name="ones32")
            nc.vector.memset(ones32, 1.0)
            ones_row = const.tile([1, 128], BF16, tag="ones_row", name="ones_row")
            nc.vector.memset(ones_row, 1.0)
            ident = const.tile([128, 128], BF16, tag="ident", name="ident")
            make_identity(nc, ident)

            loop_cm = tc.For_i(0, loop_n, 1) if loop_n > 1 else contextlib.nullcontext()
            with loop_cm:
                for b in range(B_LOC):
                    # ---- qkvT = W @ xT : [768, 1024] as 6 tiles [128, 1024] ----
                    xt_sb = [xtp.tile([128, N_TOK], BF16, tag="xt", name="xt_sb")
                             for _ in range(2)]
                    for kc in range(2):
                        nc.sync.dma_start(out=xt_sb[kc], in_=xt[b, 128 * kc:128 * (kc + 1), :])
                    qkv = [qkvp.tile([128, N_TOK], BF16, tag="qkv", name="qkv_sb")
                           for _ in range(6)]
                    for m in range(6):
                        for nh in range(2):
                            ps = mm_ps.tile([128, 512], F32, tag="mm", name="mm_ps_t")
                            for kc in range(2):
                                nc.tensor.matmul(
                                    ps,
                                    wqkv_sb[kc][:, 128 * m:128 * (m + 1)],
                                    xt_sb[kc][:, 512 * nh:512 * (nh + 1)],
                                    start=(kc == 0), stop=(kc == 1),
                                )
                            nc.vector.tensor_copy(qkv[m][:, 512 * nh:512 * (nh + 1)], ps)

                    # ---- V tiles per group: [tok 128, 4 heads x 32] ----
                    vt = [[vp.tile([128, 128], BF16, tag="v", name="v_sb")
                           for _ in range(NCHUNK)] for _ in range(2)]
                    for g in range(2):
                        for t in range(NCHUNK):
                            ps = mm_ps.tile([128, 128], BF16, tag="mm", name="mm_ps_t")
                            nc.tensor.transpose(ps, qkv[4 + g][:, 128 * t:128 * (t + 1)], ident)
                            nc.vector.tensor_copy(vt[g][t], ps)

                    aT = [atp.tile([128, N_TOK], BF16, tag="aT", name="aT_sb")
                          for _ in range(2)]
                    for g in range(2):
                        pts = [None] * NCHUNK

                        def produce(c, g=g, pts=pts):
                            # two 2-head score tiles (2 banks each) so the
                            # next chunk's matmuls never wait on this exp
                            qs, wc = _qband(c)
                            pt = ptp.tile([128, 4, 512], BF16, tag="pt", name="pt_t")
                            pts[c] = pt
                            for p in range(2):
                                sc = sc_ps.tile([128, 2, 512], F32, tag="sc", name="sc_t")
                                for jj in range(2):
                                    j = 2 * p + jj
                                    nc.tensor.matmul(
                                        sc[:, jj, :wc],
                                        qkv[2 + g][32 * j:32 * (j + 1), 128 * c:128 * (c + 1)],
                                        qkv[0 + g][32 * j:32 * (j + 1), qs:qs + wc],
                                        start=True, stop=True,
                                        tile_position=(32 * j, 0),
                                    )
                                nc.scalar.activation(pt[:, 2 * p:2 * p + 2, :wc],
                                                     sc[:, :, :wc], AF.Exp)
                                # multiply by 0/1 band mask, broadcast over heads
                                m = mask_sb[c][:, :wc]
                                mb = bass.AP(tensor=m.tensor, offset=m.offset,
                                             ap=[m.ap[0], [0, 2], m.ap[1]])
                                nc.vector.tensor_mul(pt[:, 2 * p:2 * p + 2, :wc],
                                                     pt[:, 2 * p:2 * p + 2, :wc], mb)

                        # q-quarters: accumulate out.T/denominator over chunks
                        # in a zeroed 1-bank PSUM tile (start=False throughout
                        # — correct after memset regardless of has_written)
                        produced = 0
                        for qtr in range(4):
                            h0 = 256 * qtr
                            need = max(c for c in range(NCHUNK)
                                       if _qband(c)[0] < h0 + 256)
                            while produced <= need:
                                produce(produced)
                                produced += 1
                            cons = [c for c in range(NCHUNK)
                                    if _qband(c)[0] < h0 + 256
                                    and _qband(c)[0] + _qband(c)[1] > h0]
                            od = od_ps.tile([128, 2, 256], F32, tag="od", name="od_t")
                            nc.vector.memset(od, 0.0)
                            for ci, c in enumerate(cons):
                                qs, wc = _qband(c)
                                lo = max(h0, qs)
                                hi = min(h0 + 256, qs + wc)
                                po, oo, nw = lo - qs, lo - h0, hi - lo
                                last = ci == len(cons) - 1
                                for j in range(4):
                                    nc.tensor.matmul(
                                        od[32 * j:32 * (j + 1), 0, oo:oo + nw],
                                        vt[g][c][:, 32 * j:32 * (j + 1)],
                                        pts[c][:, j, po:po + nw],
                                        start=False, stop=last,
                                        tile_position=(0, 32 * j),
                                        skip_group_check=True,
                                    )
                                    nc.tensor.matmul(
                                        od[32 * j:32 * (j + 1), 1, oo:oo + nw],
                                        ones32[:, :32],
                                        pts[c][:, j, po:po + nw],
                                        start=False, stop=last,
                                        tile_position=(0, 32 * j),
                                        skip_group_check=True,
                                    )
                            rc = drp.tile([128, 256], F32, tag="rc", name="rc_t")
                            nc.vector.reciprocal(rc, od[:, 1, :])
                            nc.vector.tensor_mul(
                                aT[g][:, h0:h0 + 256], od[:, 0, :], rc)

                    # ---- proj: y[tok,256] = aT.T @ wpT + bias ----
                    for t in range(NQT):
                        ps = mm_ps.tile([128, DIM], F32, tag="mm", name="mm_ps_t",
                                        padded_shape=[128, 512])
                        for g in range(2):
                            nc.tensor.matmul(
                                ps, aT[g][:, 128 * t:128 * (t + 1)], wp_sb[g],
                                start=(g == 0), stop=False,
                            )
                        nc.tensor.matmul(ps, ones_row, bias_sb, start=False, stop=True)
                        yt = yp.tile([128, DIM], F32, tag="y", name="y_sb")
                        nc.vector.tensor_copy(yt, ps)
                        nc.sync.dma_start(out=y[b, 128 * t:128 * (t + 1), :], in_=yt)

    nc.finalize()
    return nc


_PROGRAM = None


def _get_program():
    global _PROGRAM
    if _PROGRAM is None:
        _PROGRAM = build_program()
    return _PROGRAM


def _prep_inputs(x, w_qkv, w_proj, b_proj, mask):
    """Host-side prep: shard, transpose, cast, compact mask."""
    scale = HDIM ** -0.5
    wqkvT = np.asarray(w_qkv, np.float32).T.copy()       # [256, 768]
    wqkvT[:, :DIM] *= scale                              # fold qk scale into q
    wqkvT = wqkvT.astype(ml_dtypes.bfloat16)
    wpT = np.asarray(w_proj, np.float32).T.astype(ml_dtypes.bfloat16)
    bias = np.asarray(b_proj, np.float32).reshape(1, DIM).astype(ml_dtypes.bfloat16)

    m4 = np.asarray(mask, np.float32).reshape(N_TOK, N_TOK)  # [q, k] additive
    maskc = np.zeros((NCHUNK, 128, 512), np.float32)
    for c in range(NCHUNK):
        qs, wc = _qband(c)
        # rows: k tokens of chunk c; cols: q tokens of the band
        maskc[c, :, :wc] = (m4[qs:qs + wc, 128 * c:128 * (c + 1)] == 0.0).T
    maskc = maskc.astype(ml_dtypes.bfloat16)

    x = np.asarray(x, np.float32)
    in_maps = []
    for core in range(N_CORES):
        xs = x[core * B_LOC:(core + 1) * B_LOC]          # [4, 1024, 256]
        xtl = np.ascontiguousarray(xs.transpose(0, 2, 1)).astype(ml_dtypes.bfloat16)
        in_maps.append({"xt": xtl, "wqkvT": wqkvT, "wpT": wpT,
                        "bias": bias, "maskc": maskc})
    return in_maps


def run(inputs, trace=False):
    nc = _get_program()
    in_maps = _prep_inputs(**inputs)
    res = bass_utils.run_bass_kernel_spmd(
        nc, in_maps, core_ids=list(range(N_CORES)), trace=trace,
    )
    out = np.concatenate([res.results[i]["y"] for i in range(N_CORES)], axis=0)
    return out, res


def kernel(**inputs) -> np.ndarray:
    out, _ = run(inputs, trace=False)
    return out


# revision 20
# speedup vs baseline: 1.0174x; 1.0174x over previous
"""Sparse (sliding-window) attention Trainium2 kernel.

Problem (hardcoded shapes): B=32, N=1024 tokens on a 16x64 grid, C=256,
8 heads, head_dim=32. Local attention window: +-3 grid rows, +-5 grid
cols (7x11). y = softmax(q k^T/sqrt(d) + mask) v, projected.

Sharding: data-parallel over batch, 4 items per core on 8 cores.

Per-core algorithm (bf16 compute, fp32 PSUM accumulation):
  - qkvT[768,1024] = (w_qkv.T).T @ x.T via PE (host passes xT, w_qkv.T
    with the q part pre-scaled by d^-0.5). Layout keeps q/k per head at
    partition offsets 32j, which feeds the row-packed score matmuls.
  - scores in transposed layout ST[k_chunk=128, q_band<=512] per head:
    the band sparsity (only the +-3-grid-row window per chunk) skips
    ~2.3x of the dense score work. Heads computed concurrently via
    tile_position row packing (K=32); score tiles hold 2 heads (2 PSUM
    banks, double-buffered) so ScalarE rarely waits on the PE.
  - P = exp(ST) on ScalarE (PSUM->SBUF bf16; no max subtraction needed,
    scores are O(1)), then multiplied by a compact 0/1 band mask on
    VectorE (bf16 2x mode, mask broadcast across heads via a step-0 AP
    dim). exp(-inf additive mask) == multiplicative 0 mask here.
  - out.T[d,q] and denominators accumulate chunk-major over 256-wide
    q-quarters into a zeroed 1-bank PSUM tile with start=False
    throughout (PSUM has_written semantics make that correct after a
    DVE memset; an explicit start=True clears the WHOLE bank for the
    written partitions and would wipe sibling regions). PV uses
    col-packed matmuls (lhsT = V chunk [128,32]); the denominator uses
    lhsT = ones [128,32], which lands the row-sum pre-broadcast across
    the 32 partitions of each head, so normalization is one reciprocal
    + one elementwise multiply, already in the aT layout proj needs.
  - proj consumes the transposed attention output directly as lhsT.

Measured (8 axon-tunneled trn2 cores, For_i-loop slope timing): ~230 us
per core for the full per-core workload (4 batch items); rel err vs
fp32 reference 3.95e-3 (bf16-level).
"""

import contextlib

import numpy as np
import ml_dtypes

import concourse.bass as bass
import concourse.bacc as bacc
import concourse.mybir as mybir
import concourse.tile as tile
from concourse import bass_utils
from concourse.masks import make_identity

F32 = mybir.dt.float32
BF16 = mybir.dt.bfloat16
AF = mybir.ActivationFunctionType

H_MAP, W_MAP = 16, 64
N_TOK = H_MAP * W_MAP            # 1024
DIM = 256
HEADS = 8
HDIM = 32
B_FULL = 32
N_CORES = 8
B_LOC = B_FULL // N_CORES        # 4
NCHUNK = N_TOK // 128            # 8 k-chunks (2 grid rows each)
NQT = N_TOK // 128               # 8 q-tiles
HALF = 512


def _qband(c):
    """Valid q range (start token, width) for k-chunk c (rows 2c, 2c+1)."""
    qlo = max(0, 2 * c - 3)
    qhi = min(H_MAP - 1, 2 * c + 4)
    return qlo * W_MAP, (qhi - qlo + 1) * W_MAP


def build_program(loop_n=1):
    nc = bacc.Bacc("TRN2", target_bir_lowering=False, debug=False)

    xt_d = nc.dram_tensor("xt", [B_LOC, DIM, N_TOK], BF16, kind="ExternalInput")
    wqkvT_d = nc.dram_tensor("wqkvT", [DIM, 3 * DIM], BF16, kind="ExternalInput")
    wpT_d = nc.dram_tensor("wpT", [DIM, DIM], BF16, kind="ExternalInput")
    bias_d = nc.dram_tensor("bias", [1, DIM], BF16, kind="ExternalInput")
    maskc_d = nc.dram_tensor("maskc", [NCHUNK, 128, 512], BF16, kind="ExternalInput")
    y_d = nc.dram_tensor("y", [B_LOC, N_TOK, DIM], F32, kind="ExternalOutput")

    xt = xt_d.ap()
    y = y_d.ap()

    with tile.TileContext(nc) as tc:
        with (
            tc.tile_pool(name="const", bufs=1) as const,
            tc.tile_pool(name="xtp", bufs=4) as xtp,
            tc.tile_pool(name="qkvp", bufs=12) as qkvp,
            tc.tile_pool(name="vp", bufs=36) as vp,
            tc.tile_pool(name="ptp", bufs=12) as ptp,
            tc.tile_pool(name="atp", bufs=4) as atp,
            tc.tile_pool(name="drp", bufs=4) as drp,
            tc.tile_pool(name="yp", bufs=8) as yp,
            tc.tile_pool(name="sc_ps", bufs=2, space="PSUM") as sc_ps,
            tc.tile_pool(name="od_ps", bufs=2, space="PSUM") as od_ps,
            tc.tile_pool(name="mm_ps", bufs=2, space="PSUM") as mm_ps,
        ):
            # ---- constants ----
            wqkv_sb = [const.tile([128, 3 * DIM], BF16, tag=f"wqkv{i}", name=f"wqkv{i}")
                       for i in range(2)]
            for i in range(2):
                nc.sync.dma_start(out=wqkv_sb[i], in_=wqkvT_d.ap()[128 * i:128 * (i + 1), :])
            wp_sb = [const.tile([128, DIM], BF16, tag=f"wp{i}", name=f"wp{i}")
                     for i in range(2)]
            for i in range(2):
                nc.sync.dma_start(out=wp_sb[i], in_=wpT_d.ap()[128 * i:128 * (i + 1), :])
            bias_sb = const.tile([1, DIM], BF16, tag="bias", name="bias_sb")
            nc.sync.dma_start(out=bias_sb, in_=bias_d.ap())
            mask_sb = [const.tile([128, 512], BF16, tag=f"mask{c}", name=f"mask{c}")
                       for c in range(NCHUNK)]
            for c in range(NCHUNK):
                nc.sync.dma_start(out=mask_sb[c], in_=maskc_d.ap()[c])
            ones32 = const.tile([128, 32], BF16, tag="ones32", name="ones32")
            nc.vector.memset(ones32, 1.0)
            ones_row = const.tile([1, 128], BF16, tag="ones_row", name="ones_row")
            nc.vector.memset(ones_row, 1.0)
            ident = const.tile([128, 128], BF16, tag="ident", name="ident")
            make_identity(nc, ident)

            loop_cm = tc.For_i(0, loop_n, 1) if loop_n > 1 else contextlib.nullcontext()
            with loop_cm:
                for b in range(B_LOC):
                    # ---- qkvT = W @ xT : [768, 1024] as 6 tiles [128, 1024] ----
                    xt_sb = [xtp.tile([128, N_TOK], BF16, tag="xt", name="xt_sb")
                             for _ in range(2)]
                    for kc in range(2):
                        nc.sync.dma_start(out=xt_sb[kc], in_=xt[b, 128 * kc:128 * (kc + 1), :])
                    qkv = [qkvp.tile([128, N_TOK], BF16, tag="qkv", name="qkv_sb")
                           for _ in range(6)]
                    for m in range(6):
                        for nh in range(2):
                            ps = mm_ps.tile([128, 512], F32, tag="mm", name="mm_ps_t")
                            for kc in range(2):
                                nc.tensor.matmul(
                                    ps,
                                    wqkv_sb[kc][:, 128 * m:128 * (m + 1)],
                                    xt_sb[kc][:, 512 * nh:512 * (nh + 1)],
                                    start=(kc == 0), stop=(kc == 1),
                                )
                            nc.vector.tensor_copy(qkv[m][:, 512 * nh:512 * (nh + 1)], ps)

                    # ---- V tiles per group: [tok 128, 4 heads x 32] ----
                    vt = [[vp.tile([128, 128], BF16, tag="v", name="v_sb")
                           for _ in range(NCHUNK)] for _ in range(2)]
                    for g in range(2):
                        for t in range(NCHUNK):
                            ps = mm_ps.tile([128, 128], BF16, tag="mm", name="mm_ps_t")
                            nc.tensor.transpose(ps, qkv[4 + g][:, 128 * t:128 * (t + 1)], ident)
                            nc.vector.tensor_copy(vt[g][t], ps)

                    aT = [atp.tile([128, N_TOK], BF16, tag="aT", name="aT_sb")
                          for _ in range(2)]
                    for g in range(2):
                        pts = [None] * NCHUNK

                        def produce(c, g=g, pts=pts):
                            # two 2-head score tiles (2 banks each) so the
                            # next chunk's matmuls never wait on this exp
                            qs, wc = _qband(c)
                            pt = ptp.tile([128, 4, 512], BF16, tag="pt", name="pt_t")
                            pts[c] = pt
                            for p in range(2):
                                sc = sc_ps.tile([128, 2, 512], F32, tag="sc", name="sc_t")
                                for jj in range(2):
                                    j = 2 * p + jj
                                    nc.tensor.matmul(
                                        sc[:, jj, :wc],
                                        qkv[2 + g][32 * j:32 * (j + 1), 128 * c:128 * (c + 1)],
                                        qkv[0 + g][32 * j:32 * (j + 1), qs:qs + wc],
                                        start=True, stop=True,
                                        tile_position=(32 * j, 0),
                                    )
                                nc.scalar.activation(pt[:, 2 * p:2 * p + 2, :wc],
                                                     sc[:, :, :wc], AF.Exp)
                                # multiply by 0/1 band mask, broadcast over heads
                                m = mask_sb[c][:, :wc]
                                mb = bass.AP(tensor=m.tensor, offset=m.offset,
                                             ap=[m.ap[0], [0, 2], m.ap[1]])
                                nc.vector.tensor_mul(pt[:, 2 * p:2 * p + 2, :wc],
                                                     pt[:, 2 * p:2 * p + 2, :wc], mb)

                        # q-quarters: accumulate out.T/denominator over chunks
                        # in a zeroed 1-bank PSUM tile (start=False throughout
                        # — correct after memset regardless of has_written)
                        produced = 0
                        for qtr in range(4):
                            h0 = 256 * qtr
                            need = max(c for c in range(NCHUNK)
                                       if _qband(c)[0] < h0 + 256)
                            while produced <= need:
                                produce(produced)
                                produced += 1
                            cons = [c for c in range(NCHUNK)
                                    if _qband(c)[0] < h0 + 256
                                    and _qband(c)[0] + _qband(c)[1] > h0]
                            od = od_ps.tile([128, 2, 256], F32, tag="od", name="od_t")
                            nc.vector.memset(od, 0.0)
                            for ci, c in enumerate(cons):
                                qs, wc = _qband(c)
                                lo = max(h0, qs)
                                hi = min(h0 + 256, qs + wc)
                                po, oo, nw = lo - qs, lo - h0, hi - lo
                                last = ci == len(cons) - 1
                                for j in range(4):
                                    nc.tensor.matmul(
                                        od[32 * j:32 * (j + 1), 0, oo:oo + nw],
                                        vt[g][c][:, 32 * j:32 * (j + 1)],
                                        pts[c][:, j, po:po + nw],
                                        start=False, stop=last,
                                        tile_position=(0, 32 * j),
                                        skip_group_check=True,
                                    )
                                    nc.tensor.matmul(
                                        od[32 * j:32 * (j + 1), 1, oo:oo + nw],
                                        ones32[:, :32],
                                        pts[c][:, j, po:po + nw],
                                        start=False, stop=last,
                                        tile_position=(0, 32 * j),
                                        skip_group_check=True,
                                    )
                            rc = drp.tile([128, 256], F32, tag="rc", name="rc_t")
                            nc.vector.reciprocal(rc, od[:, 1, :])
                            nc.vector.tensor_mul(
                                aT[g][:, h0:h0 + 256], od[:, 0, :], rc)

                    # ---- proj: y[tok,256] = aT.T @ wpT + bias ----
                    for t in range(NQT):
                        ps = mm_ps.tile([128, DIM], F32, tag="mm", name="mm_ps_t",
                                        padded_shape=[128, 512])
                        for g in range(2):
                            nc.tensor.matmul(
                                ps, aT[g][:, 128 * t:128 * (t + 1)], wp_sb[g],
                                start=(g == 0), stop=False,
                            )
                        nc.tensor.matmul(ps, ones_row, bias_sb, start=False, stop=True)
                        yt = yp.tile([128, DIM], F32, tag="y", name="y_sb")
                        nc.vector.tensor_copy(yt, ps)
                        nc.sync.dma_start(out=y[b, 128 * t:128 * (t + 1), :], in_=yt)

    nc.finalize()
    return nc


_PROGRAM = None


def _get_program():
    global _PROGRAM
    if _PROGRAM is None:
        _PROGRAM = build_program()
    return _PROGRAM


def _prep_inputs(x, w_qkv, w_proj, b_proj, mask):
    """Host-side prep: shard, transpose, cast, compact mask."""
    scale = HDIM ** -0.5
    wqkvT = np.asarray(w_qkv, np.float32).T.copy()       # [256, 768]
    wqkvT[:, :DIM] *= scale                              # fold qk scale into q
    wqkvT = wqkvT.astype(ml_dtypes.bfloat16)
    wpT = np.asarray(w_proj, np.float32).T.astype(ml_dtypes.bfloat16)
    bias = np.asarray(b_proj, np.float32).reshape(1, DIM).astype(ml_dtypes.bfloat16)

    m4 = np.asarray(mask, np.float32).reshape(N_TOK, N_TOK)  # [q, k] additive
    maskc = np.zeros((NCHUNK, 128, 512), np.float32)
    for c in range(NCHUNK):
        qs, wc = _qband(c)
        # rows: k tokens of chunk c; cols: q tokens of the band
        maskc[c, :, :wc] = (m4[qs:qs + wc, 128 * c:128 * (c + 1)] == 0.0).T
    maskc = maskc.astype(ml_dtypes.bfloat16)

    x = np.asarray(x, np.float32)
    in_maps = []
    for core in range(N_CORES):
        xs = x[core * B_LOC:(core + 1) * B_LOC]          # [4, 1024, 256]
        xtl = np.ascontiguousarray(xs.transpose(0, 2, 1)).astype(ml_dtypes.bfloat16)
        in_maps.append({"xt": xtl, "wqkvT": wqkvT, "wpT": wpT,
                        "bias": bias, "maskc": maskc})
    return in_maps


def run(inputs, trace=False):
    nc = _get_program()
    in_maps = _prep_inputs(**inputs)
    res = bass_utils.run_bass_kernel_spmd(
        nc, in_maps, core_ids=list(range(N_CORES)), trace=trace,
    )
    out = np.concatenate([res.results[i]["y"] for i in range(N_CORES)], axis=0)
    return out, res


def kernel(**inputs) -> np.ndarray:
    out, _ = run(inputs, trace=False)
    return out
